# revision 66
# baseline (speedup 1.0000x reference)
"""AttnBlock (GroupNorm + single-head 1x1-conv attention + residual) on 8
Trainium2 NeuronCores, data-parallel over the batch dimension (one image per
core, weights replicated).

fp8 version: all large matmuls run in float8e4 (e4m3, max normal 240) with
MatmulPerfMode.DoubleRow -- each instruction consumes TWO K=128 tiles laid
side-by-side in the free dim of both operands at double pump rate, halving
PE streaming time vs float32r. PSUM accumulation stays fp32. Verified
numerically: rel err ~1e-3 vs the fp32 reference (gate 2e-2).

Engine economics this kernel is built around (measured):
  - ACT reloads its LUT (~1.5us) whenever the activation function changes;
    Identity needs no table. So ACT runs: Sqrt once (GN), then Exp for the
    attention est, then one Ln->Exp pair (the softmax reciprocal, batched
    for both query blocks); everything else on ACT is Identity.
  - DVE InstReciprocal costs ~6.5ns * free_size regardless of partitions
    (3.3us for 512 wide) -- hence 1/d = exp(-ln d) on ACT instead.
  - dual-ALU DVE ops (tensor_scalar mult+add) run ~2x slower than single
    ops, so the attention scale is folded into the Exp activation scale
    and all PSUM drains are single-op adds/copies.
  - GpSimd cannot touch PSUM and writes fp32 catastrophically slowly, but
    is fine for SBUF->SBUF fp8 writes (GN affine, pair-layout copies).
  - dual-fp8 LDWEIGHTS requires the [2, M] k-tile pair contiguous in SBUF:
    stationary operands (wq/wk/wp from host; k2/vT2/h2 written on device)
    use a pair-contiguous layout; moving operands keep plain layouts.

Per-core dataflow (x: [512 ch, 1024 px], fp32 in HBM):
  GN stats   : bn_stats/bn_aggr per c-tile (DVE) -> per-group sums via
               indicator matmul -> finalize (one ACT Sqrt) -> one union
               indicator matmul broadcasts (mu, rstd) back to channels ->
               h = x*a + b split across DVE/ACT/GpSimd, written as fp8
               twice (plain + pair layout)
  q/k        : DoubleRow over c-tile pairs; q on ACT (Identity+bias), k on
               DVE into the pair layout; attention scale rides the Exp
  vT         : lhsT = h2 pair, rhs = wv pair; bv folded into bproj' on the
               host so drains are pure copies (ACT/DVE split)
  S^T = k^T q: [j,i] layout, 2-bank psum pairs, exp -> fp8 est for BOTH
               query blocks; denominator ones-matmuls broadcast sum_j to
               all partitions; 1/d = exp(-ln d) on ACT
  U = v expS^T (both blocks back to back, covering the reciprocal); the
               normalization multiplies the U drain (DVE); proj + residual
               + bias fused into the drains (DVE, ACT helping on odd
               tiles); output DMA split across sync+gpsimd queues.
"""

from contextlib import ExitStack

import numpy as np
import ml_dtypes

import concourse.bass as bass
import concourse.tile as tile
from concourse import mybir
from concourse.bass_utils import run_bass_kernel_spmd
from concourse.vector_clock import ScopedClock

B, C, HH, WW = 8, 512, 32, 32
HW = HH * WW          # 1024 pixels
P = 128               # SBUF partitions
CT = C // P           # 4 channel tiles
CP = CT // 2          # 2 channel-tile pairs (DoubleRow)
JT = HW // P          # 8 pixel tiles (keys)
JP = JT // 2          # 4 pixel-tile pairs
NB = 512              # matmul moving free dim (one PSUM bank of fp32)
IB = HW // NB         # 2 query blocks
NG = 8                # groupnorm groups
GS = C // NG          # 64 channels per group
EPS = 1e-5
SC = 1.0 / float(np.sqrt(np.float32(C)))  # attention scale, applied in Exp

F32 = mybir.dt.float32
F8 = mybir.dt.float8e4          # e4m3, max normal 240
NP8 = ml_dtypes.float8_e4m3
DR = mybir.MatmulPerfMode.DoubleRow
AF = mybir.ActivationFunctionType
OP = mybir.AluOpType


class _TC(tile.TileContext):
    """This container's walrus build rejects instructions carrying more than
    one sync-wait condition. After scheduling, hoist the extra waits of every
    multi-wait instruction into single-wait EventSemaphore instructions
    inserted just before it on the same engine (semantically identical)."""

    def _split_multiwait(self):
        nc = self.nc
        for bb in nc.main_func.blocks:
            insts = bb.instructions
            out = []
            changed = False
            for inst in insts:
                si = inst.sync_info
                if si is not None and si.on_wait and len(si.on_wait) > 1:
                    waits = list(si.on_wait)
                    si.on_wait = [waits[-1]]
                    for w in waits[:-1]:
                        wi = mybir.InstEventSemaphore(
                            name=nc.get_next_instruction_name()
                        )
                        wi.engine = inst.engine
                        wi.sync_info = mybir.SyncInfo(on_wait=[w], on_update=[])
                        out.append(wi)
                    changed = True
                out.append(inst)
            if changed:
                bb.instructions = out

    def _drain_and_barrier(self, tick_clock, wait_clock):
        nc = self.nc
        drain_inst = nc.sync.drain()
        wait_clock.add_sem_waits(
            drain_inst.ins, ScopedClock({None: tick_clock.global_clock})
        )
        self._split_multiwait()
        popped = nc._tile_sem_poison_stack.pop()
        assert popped is self._sem_poison


def _build():
    nc = bass.Bass()
    x = nc.dram_tensor("x", [C, HW], F32, kind="ExternalInput")
    wq = nc.dram_tensor("wq2", [P, CP * CT * 2 * P], F8, kind="ExternalInput")
    wk = nc.dram_tensor("wk2", [P, CP * CT * 2 * P], F8, kind="ExternalInput")
    wp = nc.dram_tensor("wp2", [P, CP * CT * 2 * P], F8, kind="ExternalInput")
    wv = nc.dram_tensor("wvT", [C, C], F8, kind="ExternalInput")
    SCAT = 5 * CT + CT * NG + CT * P
    smallcat = nc.dram_tensor("smallcat", [P, SCAT], F32, kind="ExternalInput")
    out = nc.dram_tensor("out", [C, HW], F32, kind="ExternalOutput")

    with _TC(nc) as tc, ExitStack() as ctx:
        big = ctx.enter_context(tc.tile_pool(name="big", bufs=1))
        small = ctx.enter_context(tc.tile_pool(name="small", bufs=1))
        tmp = ctx.enter_context(tc.tile_pool(name="tmp", bufs=4))
        # singles: 4 x 1-bank [P, NB] (q/k/v/U/proj/denom + GN smalls);
        # pairs: 2 x 2-bank [P, 2, NB] for S^T so exp drains two banks/op
        ps_pool = ctx.enter_context(tc.tile_pool(name="ps", bufs=4, space="PSUM"))
        ps_pair = ctx.enter_context(tc.tile_pool(name="psp", bufs=2, space="PSUM"))
        outp = ctx.enter_context(tc.tile_pool(name="outp", bufs=6))

        # ---- loads: x first at full bandwidth, weights behind it ----
        sc_sb = small.tile([P, SCAT], F32, tag="smallcat")
        xsb = big.tile([P, CT, HW], F32, tag="xsb")
        xr = x.rearrange("(t p) i -> p t i", p=P)
        # ALL input DMAs chained on the sync queue: each dma_start stripes
        # across all 16 hardware queues, and queue FIFO order serializes the
        # transfers -- so x tile 0 lands at ~1.5us (full 358GB/s to itself)
        # instead of sharing bandwidth with everything else for 10us. Each
        # tile's GN chain starts the moment its tile lands.
        nc.sync.dma_start(out=xsb[:, 0, :], in_=xr[:, 0, :])
        nc.sync.dma_start(out=xsb[:, 1, :], in_=xr[:, 1, :])
        nc.scalar.dma_start(out=sc_sb[:], in_=smallcat[:])
        bq_sb = sc_sb[:, 0 * CT : 1 * CT]
        bk_sb = sc_sb[:, 1 * CT : 2 * CT]
        bp_sb = sc_sb[:, 2 * CT : 3 * CT]          # bproj + wproj@bv
        gs_sb = sc_sb[:, 3 * CT : 4 * CT]
        gb_sb = sc_sb[:, 4 * CT : 5 * CT]
        gmat_sb = sc_sb[:, 5 * CT : 5 * CT + CT * NG].rearrange(
            "p (t g) -> p t g", t=CT
        )
        hmat_sb = sc_sb[:, 5 * CT + CT * NG :]
        wq_sb = big.tile([P, CP, CT, 2, P], F8, tag="wq")
        nc.sync.dma_start(
            out=wq_sb[:], in_=wq.rearrange("p (c t h o) -> p c t h o", c=CP, t=CT, h=2)
        )
        wk_sb = big.tile([P, CP, CT, 2, P], F8, tag="wk")
        nc.sync.dma_start(
            out=wk_sb[:], in_=wk.rearrange("p (c t h o) -> p c t h o", c=CP, t=CT, h=2)
        )
        nc.scalar.dma_start(out=xsb[:, 2, :], in_=xr[:, 2, :])
        nc.gpsimd.dma_start(out=xsb[:, 3, :], in_=xr[:, 3, :])
        wv_sb = big.tile([P, CT, C], F8, tag="wv")
        nc.sync.dma_start(out=wv_sb[:], in_=wv.rearrange("(t p) o -> p t o", p=P))
        wp_sb = big.tile([P, CP, CT, 2, P], F8, tag="wp")
        nc.sync.dma_start(
            out=wp_sb[:], in_=wp.rearrange("p (c t h o) -> p c t h o", c=CP, t=CT, h=2)
        )

        # ---- constants ----
        ones_f32 = small.tile([P, 2 * P], F32, tag="ones32")
        nc.vector.memset(ones_f32[:], 1.0)
        ones8 = small.tile([P, 2, P], F8, tag="ones8")
        nc.vector.tensor_copy(
            out=ones8[:], in_=ones_f32[:].rearrange("p (t o) -> p t o", t=2)
        )
        eps_sb = small.tile([P, 1], F32, tag="eps")
        nc.vector.memset(eps_sb[:], EPS)
        # per-tile (mu, rstd) column pairs on the 8 group partitions; rows
        # 8..127 stay zero for the K=128 broadcast matmuls
        gsf = small.tile([P, CT * 2], F32, tag="gsf")
        nc.vector.memset(gsf[:], 0.0)
        # ACT warm: load the Sqrt table (first real ACT table use is GN)
        warm = tmp.tile([1, 1], F32, tag="actwarm")
        nc.scalar.activation(out=warm[:], in_=ones_f32[0:1, 0:1], func=AF.Sqrt)

        est = big.tile([P, JT, HW], F8, tag="est")  # exp(S^T/sqrt(c)), [j, i]
        # PE pstate warmup: dummy DoubleRow matmuls on constant/garbage SBUF
        # during the x DMA wait keep the tensor clock ramped. est[:, 6:8, :]
        # is not written before S^T; the result psum is never read.
        ps_w = ps_pool.tile([P, NB], F32, tag="mmps", name="warmmm")
        for _ in range(3):
            nc.tensor.matmul(
                ps_w[:], ones8[:], est[:, 6:8, 0:NB],
                start=True, stop=True, perf_mode=DR,
            )

        # ---- groupnorm, fully per-tile: groups {2t, 2t+1} live entirely
        # in channel tile t, so each tile runs stats -> finalize -> affine
        # the moment its (serially ordered) x DMA lands. Each tile gets its
        # own zeroed gsf_t: rows outside its two groups hold finite garbage
        # that its indicator matmul multiplies by zero.
        mom = small.tile([P, CT, 2], F32, tag="mom")  # (mean, E[x^2]) per ch
        bns = small.tile([P, CT, 2, 6], F32, tag="bns")
        sc = tmp.tile([P, CT, 4], F32, tag="gnsc")
        ab = small.tile([P, CT, 2], F32, tag="ab")
        hsb = big.tile([P, CT, HW], F8, tag="hsb")
        h2 = big.tile([P, CP, JT, 2, P], F8, tag="h2")
        for t in range(CT):
            for c in range(2):
                nc.vector.bn_stats(
                    out=bns[:, t, c, :],
                    in_=xsb[:, t, c * NB : (c + 1) * NB],
                )
            nc.vector.bn_aggr(out=mom[:, t, :], in_=bns[:, t, :, :])
            # E[x^2] = var + mean^2, in place on the second column
            nc.vector.scalar_tensor_tensor(
                out=mom[:, t, 1:2], in0=mom[:, t, 0:1], scalar=mom[:, t, 0:1],
                in1=mom[:, t, 1:2], op0=OP.mult, op1=OP.add,
            )
            ps_g = ps_pool.tile([NG, 2], F32, tag="mmps", name="psg")
            nc.tensor.matmul(
                ps_g[:], gmat_sb[:, t, :], mom[:, t, :], start=True, stop=True
            )
            # finalize on partitions 0:8: mu_g = S1/GS, negvar = mu^2-S2/GS,
            # rstd = 1/sqrt(-negvar+eps) via Sqrt(scale=-1), warm table
            nc.vector.tensor_scalar_mul(sc[0:NG, t, 0:2], ps_g[0:NG, :], 1.0 / GS)
            nc.vector.scalar_tensor_tensor(
                out=sc[0:NG, t, 2:3], in0=sc[0:NG, t, 0:1], scalar=sc[0:NG, t, 0:1],
                in1=sc[0:NG, t, 1:2], op0=OP.mult, op1=OP.subtract,
            )
            nc.scalar.activation(
                out=sc[0:NG, t, 2:3], in_=sc[0:NG, t, 2:3],
                func=AF.Sqrt, bias=eps_sb[0:NG, 0:1], scale=-1.0,
            )
            gsf_t = gsf[:, 2 * t : 2 * t + 2]
            nc.vector.tensor_copy(out=gsf_t[0:NG, 0:1], in_=sc[0:NG, t, 0:1])
            nc.vector.reciprocal(out=gsf_t[0:NG, 1:2], in_=sc[0:NG, t, 2:3])

            ps_b = ps_pool.tile([P, 2], F32, tag="mmps", name="psb")
            nc.tensor.matmul(
                ps_b[:], hmat_sb[:].rearrange("g (t q) -> g t q", t=CT)[:, t, :],
                gsf_t[:], start=True, stop=True,
            )
            # ab via ACT Identity chains (per-partition scale/bias, no
            # tables) -- keeps DVE free to pace the bn_stats pipeline
            nc.scalar.activation(
                out=ab[:, t, 0:1], in_=ps_b[:, 1:2], func=AF.Identity,
                scale=gs_sb[:, t : t + 1],
            )
            nc.scalar.activation(
                out=ab[:, t, 1:2], in_=ps_b[:, 0:1], func=AF.Identity,
                scale=ab[:, t, 0:1],
            )
            nc.scalar.activation(
                out=ab[:, t, 1:2], in_=ab[:, t, 1:2], func=AF.Identity,
                scale=-1.0, bias=gb_sb[:, t : t + 1],
            )
            if t == 0:
                nc.scalar.activation(
                    out=hsb[:, t, :], in_=xsb[:, t, :], func=AF.Identity,
                    scale=ab[:, t, 0:1], bias=ab[:, t, 1:2],
                )
            else:
                nc.gpsimd.tensor_scalar(
                    out=hsb[:, t, :], in0=xsb[:, t, :],
                    scalar1=ab[:, t, 0:1], scalar2=ab[:, t, 1:2],
                    op0=OP.mult, op1=OP.add,
                )
        # keep the tensor clock ramped through the affine wait
        for _ in range(4):
            nc.tensor.matmul(
                ps_w[:], ones8[:], est[:, 6:8, 0:NB],
                start=True, stop=True, perf_mode=DR,
            )
        # Exp table preload on ACT, hidden under the q/k matmul phase
        nc.scalar.activation(out=warm[:], in_=ones_f32[0:1, 0:1], func=AF.Exp)
        # (h2 copies follow on GpSimd; only needed by the vT matmuls)
        for t in range(CT):
            nc.gpsimd.tensor_scalar(
                out=h2[:, t // 2, :, t % 2, :], in0=xsb[:, t, :],
                scalar1=ab[:, t, 0:1], scalar2=ab[:, t, 1:2],
                op0=OP.mult, op1=OP.add,
            )

        # ---- q / k projections, DoubleRow over c-tile pairs ----
        # q plain [c, hw] (moving in S^T); k pair-contiguous (stationary)
        q_sb = big.tile([P, CT, HW], F8, tag="q")
        k2 = big.tile([P, CP, JT, 2, P], F8, tag="k2")
        for ot in range(CT):
            for ib in range(IB):
                isl = slice(ib * NB, (ib + 1) * NB)
                psq = ps_pool.tile([P, NB], F32, tag="mmps")
                for cp in range(CP):
                    nc.tensor.matmul(
                        psq[:], wq_sb[:, cp, ot, :, :],
                        hsb[:, 2 * cp : 2 * cp + 2, isl],
                        start=(cp == 0), stop=(cp == CP - 1), perf_mode=DR,
                    )
                nc.scalar.activation(
                    out=q_sb[:, ot, isl], in_=psq[:],
                    func=AF.Identity, bias=bq_sb[:, ot : ot + 1],
                )
                psk = ps_pool.tile([P, NB], F32, tag="mmps")
                for cp in range(CP):
                    nc.tensor.matmul(
                        psk[:], wk_sb[:, cp, ot, :, :],
                        hsb[:, 2 * cp : 2 * cp + 2, isl],
                        start=(cp == 0), stop=(cp == CP - 1), perf_mode=DR,
                    )
                nc.vector.tensor_scalar_add(
                    k2[:, ot // 2, 4 * ib : 4 * ib + 4, ot % 2, :],
                    psk[:], bk_sb[:, ot : ot + 1],
                )

        # ---- vT projection; pure copy drains split ACT/DVE ----
        vT2 = big.tile([P, JP, CT, 2, P], F8, tag="vT2")
        for jt in range(JT):
            psv = ps_pool.tile([P, NB], F32, tag="mmps")
            for cp in range(CP):
                nc.tensor.matmul(
                    psv[:], h2[:, cp, jt, :, :], wv_sb[:, 2 * cp : 2 * cp + 2, :],
                    start=(cp == 0), stop=(cp == CP - 1), perf_mode=DR,
                )
            if jt % 2 == 0:
                nc.scalar.activation(
                    out=vT2[:, jt // 2, :, jt % 2, :], in_=psv[:], func=AF.Identity
                )
            else:
                nc.vector.tensor_copy(out=vT2[:, jt // 2, :, jt % 2, :], in_=psv[:])

        # ---- S^T + exp + denominator for BOTH query blocks ----
        rep = big.tile([P, HW], F32, tag="rep")
        u_sb = big.tile([P, CT, HW], F8, tag="u")
        ps_ds = []
        for ib in range(IB):
            isl = slice(ib * NB, (ib + 1) * NB)
            ps_d = ps_pool.tile([P, NB], F32, tag="mmps", name=f"psd{ib}")
            for jp in range(JP):
                pss = ps_pair.tile([P, 2, NB], F32, tag="sps")
                for h in range(2):
                    for cp in range(CP):
                        nc.tensor.matmul(
                            pss[:, h, :], k2[:, cp, 2 * jp + h, :, :],
                            q_sb[:, 2 * cp : 2 * cp + 2, isl],
                            start=(cp == 0), stop=(cp == CP - 1), perf_mode=DR,
                        )
                # attention scale rides the exp; fp8 est for both blocks
                nc.scalar.activation(
                    out=est[:, 2 * jp : 2 * jp + 2, isl], in_=pss[:],
                    func=AF.Exp, scale=SC,
                )
                # denom[i] += est pair, broadcast to all 128 partitions
                nc.tensor.matmul(
                    ps_d[:], ones8[:], est[:, 2 * jp : 2 * jp + 2, isl],
                    start=(jp == 0), stop=(jp == JP - 1), perf_mode=DR,
                )
            ps_ds.append(ps_d)
        # split reciprocals across engines so they run in parallel: block 0
        # on DVE (starts right after den0, overlapped by the S1 matmuls),
        # block 1 as exp(-ln d) on ACT right after the est exps
        nc.vector.reciprocal(out=rep[:, 0:NB], in_=ps_ds[0][:])
        nc.scalar.activation(out=rep[:, NB:HW], in_=ps_ds[1][:], func=AF.Ln)
        nc.scalar.activation(
            out=rep[:, NB:HW], in_=rep[:, NB:HW], func=AF.Exp, scale=-1.0
        )

        # ---- per block: U (covers the reciprocal), then proj + out DMA ----
        outr = out.rearrange("(t p) i -> p t i", p=P)
        for ib in range(IB):
            isl = slice(ib * NB, (ib + 1) * NB)
            for ct in range(CT):
                psu = ps_pool.tile([P, NB], F32, tag="mmps", name="psu")
                for jp in range(JP):
                    nc.tensor.matmul(
                        psu[:], vT2[:, jp, ct, :, :],
                        est[:, 2 * jp : 2 * jp + 2, isl],
                        start=(jp == 0), stop=(jp == JP - 1), perf_mode=DR,
                    )
                nc.vector.tensor_tensor(
                    out=u_sb[:, ct, isl], in0=psu[:], in1=rep[:, isl], op=OP.mult
                )
            for ot in range(CT):
                psp = ps_pool.tile([P, NB], F32, tag="mmps")
                for cp in range(CP):
                    nc.tensor.matmul(
                        psp[:], wp_sb[:, cp, ot, :, :],
                        u_sb[:, 2 * cp : 2 * cp + 2, isl],
                        start=(cp == 0), stop=(cp == CP - 1), perf_mode=DR,
                    )
                ot_t = outp.tile([P, NB], F32, tag="out", name="ot_t")
                if ot % 2 == 0:
                    nc.vector.scalar_tensor_tensor(
                        out=ot_t[:], in0=psp[:], scalar=bp_sb[:, ot : ot + 1],
                        in1=xsb[:, ot, isl], op0=OP.add, op1=OP.add,
                    )
                else:
                    # ACT (Identity, no table) + DVE so drains keep pace
                    yb = outp.tile([P, NB], F32, tag="out", name="yb")
                    nc.scalar.activation(
                        out=yb[:], in_=psp[:], func=AF.Identity,
                        bias=bp_sb[:, ot : ot + 1],
                    )
                    nc.vector.tensor_tensor(
                        out=ot_t[:], in0=yb[:], in1=xsb[:, ot, isl], op=OP.add
                    )
                # one dma_start per tile (it stripes across all 16 queues),
                # all on the sync queue: fewer completion semaphores for the
                # final barrier to sweep
                nc.sync.dma_start(out=outr[:, ot, isl], in_=ot_t[:])
    return nc


_NC = None


def _get_nc():
    global _NC
    if _NC is None:
        _NC = _build()
    return _NC


def _prep_inputs(x, gn_scale, gn_bias, wq, bq, wk, bk, wv, bv, wproj, bproj):
    f = np.float32
    x = np.ascontiguousarray(x, dtype=f).reshape(B, C, HW)

    def t8(w):  # [o, c] -> [c, o] fp8
        return np.ascontiguousarray(np.asarray(w, dtype=f).T).astype(NP8)

    def w2(w):  # [o, c] -> [p, (cp ot h o')] dual-fp8 ldweights layout
        wT = np.asarray(w, dtype=f).T  # [c, o]
        a = wT.reshape(CP, 2, P, CT, P)  # [cp, h, p, ot, o']
        a = a.transpose(2, 0, 3, 1, 4).reshape(P, CP * CT * 2 * P)
        return np.ascontiguousarray(a).astype(NP8)

    def pt(v):  # [512] -> [128, 4] with v[t*128 + p] at [p, t]
        return np.ascontiguousarray(np.asarray(v, dtype=f).reshape(CT, P).T)

    pidx = np.arange(P)[:, None]
    tidx = np.arange(CT)[None, :]
    grp = 2 * tidx + pidx // GS  # [128, 4] group id per (p, t)
    gmat = np.zeros((P, CT, NG), f)
    hmat = np.zeros((P, CT, P), f)
    for t in range(CT):
        gmat[pidx[:, 0], t, grp[:, t]] = 1.0
        hmat[grp[:, t], t, pidx[:, 0]] = 1.0

    bp_eff = np.asarray(bproj, f) + np.asarray(wproj, f) @ np.asarray(bv, f)
    smallcat = np.concatenate(
        [
            pt(bq), pt(bk), pt(bp_eff), pt(gn_scale), pt(gn_bias),
            gmat.reshape(P, CT * NG), hmat.reshape(P, CT * P),
        ],
        axis=1,
    )
    shared = {
        "wq2": w2(wq), "wk2": w2(wk), "wp2": w2(wproj), "wvT": t8(wv),
        "smallcat": np.ascontiguousarray(smallcat),
    }
    return [dict(shared, x=np.ascontiguousarray(x[b])) for b in range(B)]


def _run(inputs, **kw):
    nc = _get_nc()
    in_maps = _prep_inputs(**inputs)
    return run_bass_kernel_spmd(nc, in_maps, core_ids=list(range(B)), **kw)


def kernel(**inputs) -> np.ndarray:
    res = _run(inputs)
    out = np.stack([res.results[b]["out"] for b in range(B)])
    return out.reshape(B, C, HH, WW).astype(np.float32)


# revision 67
# speedup vs baseline: 1.0869x; 1.0869x over previous
"""AttnBlock (GroupNorm + single-head 1x1-conv attention + residual) on 8
Trainium2 NeuronCores, data-parallel over the batch dimension (one image per
core, weights replicated).

fp8 version: all large matmuls run in float8e4 (e4m3, max normal 240) with
MatmulPerfMode.DoubleRow -- each instruction consumes TWO K=128 tiles laid
side-by-side in the free dim of both operands at double pump rate, halving
PE streaming time vs float32r. PSUM accumulation stays fp32. Verified
numerically: rel err ~1e-3 vs the fp32 reference (gate 2e-2).

Engine economics this kernel is built around (measured):
  - ACT reloads its LUT (~1.5us) whenever the activation function changes;
    Identity needs no table. So ACT runs: Sqrt once (GN), then Exp for the
    attention est, then one Ln->Exp pair (the softmax reciprocal, batched
    for both query blocks); everything else on ACT is Identity.
  - DVE InstReciprocal costs ~6.5ns * free_size regardless of partitions
    (3.3us for 512 wide) -- hence 1/d = exp(-ln d) on ACT instead.
  - dual-ALU DVE ops (tensor_scalar mult+add) run ~2x slower than single
    ops, so the attention scale is folded into the Exp activation scale
    and all PSUM drains are single-op adds/copies.
  - GpSimd cannot touch PSUM and writes fp32 catastrophically slowly, but
    is fine for SBUF->SBUF fp8 writes (GN affine, pair-layout copies).
  - dual-fp8 LDWEIGHTS requires the [2, M] k-tile pair contiguous in SBUF:
    stationary operands (wq/wk/wp from host; k2/vT2/h2 written on device)
    use a pair-contiguous layout; moving operands keep plain layouts.

Per-core dataflow (x: [512 ch, 1024 px], fp32 in HBM):
  GN stats   : bn_stats/bn_aggr per c-tile (DVE) -> per-group sums via
               indicator matmul -> finalize (one ACT Sqrt) -> one union
               indicator matmul broadcasts (mu, rstd) back to channels ->
               h = x*a + b split across DVE/ACT/GpSimd, written as fp8
               twice (plain + pair layout)
  q/k        : DoubleRow over c-tile pairs; q on ACT (Identity+bias), k on
               DVE into the pair layout; attention scale rides the Exp
  vT         : lhsT = h2 pair, rhs = wv pair; bv folded into bproj' on the
               host so drains are pure copies (ACT/DVE split)
  S^T = k^T q: [j,i] layout, 2-bank psum pairs, exp -> fp8 est for BOTH
               query blocks; denominator ones-matmuls broadcast sum_j to
               all partitions; 1/d = exp(-ln d) on ACT
  U = v expS^T (both blocks back to back, covering the reciprocal); the
               normalization multiplies the U drain (DVE); proj + residual
               + bias fused into the drains (DVE, ACT helping on odd
               tiles); output DMA split across sync+gpsimd queues.
"""

from contextlib import ExitStack

import numpy as np
import ml_dtypes

import concourse.bass as bass
import concourse.tile as tile
from concourse import mybir
from concourse.bass_utils import run_bass_kernel_spmd
from concourse.vector_clock import ScopedClock

B, C, HH, WW = 8, 512, 32, 32
HW = HH * WW          # 1024 pixels
P = 128               # SBUF partitions
CT = C // P           # 4 channel tiles
CP = CT // 2          # 2 channel-tile pairs (DoubleRow)
JT = HW // P          # 8 pixel tiles (keys)
JP = JT // 2          # 4 pixel-tile pairs
NB = 512              # matmul moving free dim (one PSUM bank of fp32)
IB = HW // NB         # 2 query blocks
NG = 8                # groupnorm groups
GS = C // NG          # 64 channels per group
EPS = 1e-5
SC = 1.0 / float(np.sqrt(np.float32(C)))  # attention scale, applied in Exp

F32 = mybir.dt.float32
F8 = mybir.dt.float8e4          # e4m3, max normal 240
NP8 = ml_dtypes.float8_e4m3
DR = mybir.MatmulPerfMode.DoubleRow
AF = mybir.ActivationFunctionType
OP = mybir.AluOpType


class _TC(tile.TileContext):
    """This container's walrus build rejects instructions carrying more than
    one sync-wait condition. After scheduling, hoist the extra waits of every
    multi-wait instruction into single-wait EventSemaphore instructions
    inserted just before it on the same engine (semantically identical)."""

    def _split_multiwait(self):
        nc = self.nc
        for bb in nc.main_func.blocks:
            insts = bb.instructions
            out = []
            changed = False
            for inst in insts:
                si = inst.sync_info
                if si is not None and si.on_wait and len(si.on_wait) > 1:
                    waits = list(si.on_wait)
                    si.on_wait = [waits[-1]]
                    for w in waits[:-1]:
                        wi = mybir.InstEventSemaphore(
                            name=nc.get_next_instruction_name()
                        )
                        wi.engine = inst.engine
                        wi.sync_info = mybir.SyncInfo(on_wait=[w], on_update=[])
                        out.append(wi)
                    changed = True
                out.append(inst)
            if changed:
                bb.instructions = out

    def _drain_and_barrier(self, tick_clock, wait_clock):
        nc = self.nc
        drain_inst = nc.sync.drain()
        wait_clock.add_sem_waits(
            drain_inst.ins, ScopedClock({None: tick_clock.global_clock})
        )
        self._split_multiwait()
        popped = nc._tile_sem_poison_stack.pop()
        assert popped is self._sem_poison


def _build():
    nc = bass.Bass()
    x = nc.dram_tensor("x", [C, HW], F32, kind="ExternalInput")
    wq = nc.dram_tensor("wq2", [P, CP * CT * 2 * P], F8, kind="ExternalInput")
    wk = nc.dram_tensor("wk2", [P, CP * CT * 2 * P], F8, kind="ExternalInput")
    wp = nc.dram_tensor("wp2", [P, CP * CT * 2 * P], F8, kind="ExternalInput")
    wv = nc.dram_tensor("wvT", [C, C], F8, kind="ExternalInput")
    SCAT = 5 * CT + CT * NG + CT * P
    smallcat = nc.dram_tensor("smallcat", [P, SCAT], F32, kind="ExternalInput")
    out = nc.dram_tensor("out", [C, HW], F32, kind="ExternalOutput")

    with _TC(nc) as tc, ExitStack() as ctx:
        big = ctx.enter_context(tc.tile_pool(name="big", bufs=1))
        small = ctx.enter_context(tc.tile_pool(name="small", bufs=1))
        tmp = ctx.enter_context(tc.tile_pool(name="tmp", bufs=4))
        # singles: 4 x 1-bank [P, NB] (q/k/v/U/proj/denom + GN smalls);
        # pairs: 2 x 2-bank [P, 2, NB] for S^T so exp drains two banks/op
        ps_pool = ctx.enter_context(tc.tile_pool(name="ps", bufs=4, space="PSUM"))
        ps_pair = ctx.enter_context(tc.tile_pool(name="psp", bufs=2, space="PSUM"))
        outp = ctx.enter_context(tc.tile_pool(name="outp", bufs=6))

        # ---- loads: x first at full bandwidth, weights behind it ----
        sc_sb = small.tile([P, SCAT], F32, tag="smallcat")
        xsb = big.tile([P, CT, HW], F32, tag="xsb")
        xr = x.rearrange("(t p) i -> p t i", p=P)
        # ALL input DMAs chained on the sync queue: each dma_start stripes
        # across all 16 hardware queues, and queue FIFO order serializes the
        # transfers -- so x tile 0 lands at ~1.5us (full 358GB/s to itself)
        # instead of sharing bandwidth with everything else for 10us. Each
        # tile's GN chain starts the moment its tile lands.
        nc.sync.dma_start(out=xsb[:, 0, :], in_=xr[:, 0, :])
        nc.sync.dma_start(out=xsb[:, 1, :], in_=xr[:, 1, :])
        nc.sync.dma_start(out=sc_sb[:], in_=smallcat[:])
        bq_sb = sc_sb[:, 0 * CT : 1 * CT]
        bk_sb = sc_sb[:, 1 * CT : 2 * CT]
        bp_sb = sc_sb[:, 2 * CT : 3 * CT]          # bproj + wproj@bv
        gs_sb = sc_sb[:, 3 * CT : 4 * CT]
        gb_sb = sc_sb[:, 4 * CT : 5 * CT]
        gmat_sb = sc_sb[:, 5 * CT : 5 * CT + CT * NG].rearrange(
            "p (t g) -> p t g", t=CT
        )
        hmat_sb = sc_sb[:, 5 * CT + CT * NG :]
        wq_sb = big.tile([P, CP, CT, 2, P], F8, tag="wq")
        nc.sync.dma_start(
            out=wq_sb[:], in_=wq.rearrange("p (c t h o) -> p c t h o", c=CP, t=CT, h=2)
        )
        wk_sb = big.tile([P, CP, CT, 2, P], F8, tag="wk")
        nc.sync.dma_start(
            out=wk_sb[:], in_=wk.rearrange("p (c t h o) -> p c t h o", c=CP, t=CT, h=2)
        )
        nc.sync.dma_start(out=xsb[:, 2, :], in_=xr[:, 2, :])
        nc.sync.dma_start(out=xsb[:, 3, :], in_=xr[:, 3, :])
        wv_sb = big.tile([P, CT, C], F8, tag="wv")
        nc.sync.dma_start(out=wv_sb[:], in_=wv.rearrange("(t p) o -> p t o", p=P))
        wp_sb = big.tile([P, CP, CT, 2, P], F8, tag="wp")
        nc.sync.dma_start(
            out=wp_sb[:], in_=wp.rearrange("p (c t h o) -> p c t h o", c=CP, t=CT, h=2)
        )

        # ---- constants ----
        ones_f32 = small.tile([P, 2 * P], F32, tag="ones32")
        nc.vector.memset(ones_f32[:], 1.0)
        ones8 = small.tile([P, 2, P], F8, tag="ones8")
        nc.vector.tensor_copy(
            out=ones8[:], in_=ones_f32[:].rearrange("p (t o) -> p t o", t=2)
        )
        eps_sb = small.tile([P, 1], F32, tag="eps")
        nc.vector.memset(eps_sb[:], EPS)
        # per-tile (mu, rstd) column pairs on the 8 group partitions; rows
        # 8..127 stay zero for the K=128 broadcast matmuls
        gsf = small.tile([P, CT * 2], F32, tag="gsf")
        nc.vector.memset(gsf[:], 0.0)
        # ACT warm: load the Sqrt table (first real ACT table use is GN)
        warm = tmp.tile([1, 1], F32, tag="actwarm")
        nc.scalar.activation(out=warm[:], in_=ones_f32[0:1, 0:1], func=AF.Sqrt)

        est = big.tile([P, JT, HW], F8, tag="est")  # exp(S^T/sqrt(c)), [j, i]
        # PE pstate warmup: dummy DoubleRow matmuls on constant/garbage SBUF
        # during the x DMA wait keep the tensor clock ramped. est[:, 6:8, :]
        # is not written before S^T; the result psum is never read.
        ps_w = ps_pool.tile([P, NB], F32, tag="mmps", name="warmmm")
        for _ in range(3):
            nc.tensor.matmul(
                ps_w[:], ones8[:], est[:, 6:8, 0:NB],
                start=True, stop=True, perf_mode=DR,
            )

        # ---- groupnorm, fully per-tile: groups {2t, 2t+1} live entirely
        # in channel tile t, so each tile runs stats -> finalize -> affine
        # the moment its (serially ordered) x DMA lands. Each tile gets its
        # own zeroed gsf_t: rows outside its two groups hold finite garbage
        # that its indicator matmul multiplies by zero.
        mom = small.tile([P, CT, 2], F32, tag="mom")  # (mean, E[x^2]) per ch
        bns = small.tile([P, CT, 2, 6], F32, tag="bns")
        sc = tmp.tile([P, CT, 4], F32, tag="gnsc")
        ab = small.tile([P, CT, 2], F32, tag="ab")
        hsb = big.tile([P, CT, HW], F8, tag="hsb")
        h2 = big.tile([P, CP, JT, 2, P], F8, tag="h2")
        for t in range(CT):
            for c in range(2):
                nc.vector.bn_stats(
                    out=bns[:, t, c, :],
                    in_=xsb[:, t, c * NB : (c + 1) * NB],
                )
            nc.vector.bn_aggr(out=mom[:, t, :], in_=bns[:, t, :, :])
            # E[x^2] = var + mean^2, in place on the second column
            nc.vector.scalar_tensor_tensor(
                out=mom[:, t, 1:2], in0=mom[:, t, 0:1], scalar=mom[:, t, 0:1],
                in1=mom[:, t, 1:2], op0=OP.mult, op1=OP.add,
            )
            ps_g = ps_pool.tile([NG, 2], F32, tag="mmps", name="psg")
            nc.tensor.matmul(
                ps_g[:], gmat_sb[:, t, :], mom[:, t, :], start=True, stop=True
            )
            # finalize on partitions 0:8: mu_g = S1/GS, negvar = mu^2-S2/GS,
            # rstd = 1/sqrt(-negvar+eps) via Sqrt(scale=-1), warm table
            nc.vector.tensor_scalar_mul(sc[0:NG, t, 0:2], ps_g[0:NG, :], 1.0 / GS)
            nc.vector.scalar_tensor_tensor(
                out=sc[0:NG, t, 2:3], in0=sc[0:NG, t, 0:1], scalar=sc[0:NG, t, 0:1],
                in1=sc[0:NG, t, 1:2], op0=OP.mult, op1=OP.subtract,
            )
            nc.scalar.activation(
                out=sc[0:NG, t, 2:3], in_=sc[0:NG, t, 2:3],
                func=AF.Sqrt, bias=eps_sb[0:NG, 0:1], scale=-1.0,
            )
            gsf_t = gsf[:, 2 * t : 2 * t + 2]
            nc.vector.tensor_copy(out=gsf_t[0:NG, 0:1], in_=sc[0:NG, t, 0:1])
            nc.vector.reciprocal(out=gsf_t[0:NG, 1:2], in_=sc[0:NG, t, 2:3])

            ps_b = ps_pool.tile([P, 2], F32, tag="mmps", name="psb")
            nc.tensor.matmul(
                ps_b[:], hmat_sb[:].rearrange("g (t q) -> g t q", t=CT)[:, t, :],
                gsf_t[:], start=True, stop=True,
            )
            # ab via ACT Identity chains (per-partition scale/bias, no
            # tables) -- keeps DVE free to pace the bn_stats pipeline
            nc.scalar.activation(
                out=ab[:, t, 0:1], in_=ps_b[:, 1:2], func=AF.Identity,
                scale=gs_sb[:, t : t + 1],
            )
            nc.scalar.activation(
                out=ab[:, t, 1:2], in_=ps_b[:, 0:1], func=AF.Identity,
                scale=ab[:, t, 0:1],
            )
            nc.scalar.activation(
                out=ab[:, t, 1:2], in_=ab[:, t, 1:2], func=AF.Identity,
                scale=-1.0, bias=gb_sb[:, t : t + 1],
            )
            if t == 0:
                nc.scalar.activation(
                    out=hsb[:, t, :], in_=xsb[:, t, :], func=AF.Identity,
                    scale=ab[:, t, 0:1], bias=ab[:, t, 1:2],
                )
            else:
                nc.gpsimd.tensor_scalar(
                    out=hsb[:, t, :], in0=xsb[:, t, :],
                    scalar1=ab[:, t, 0:1], scalar2=ab[:, t, 1:2],
                    op0=OP.mult, op1=OP.add,
                )
        # keep the tensor clock ramped through the affine wait
        for _ in range(4):
            nc.tensor.matmul(
                ps_w[:], ones8[:], est[:, 6:8, 0:NB],
                start=True, stop=True, perf_mode=DR,
            )
        # Exp table preload on ACT, hidden under the q/k matmul phase
        nc.scalar.activation(out=warm[:], in_=ones_f32[0:1, 0:1], func=AF.Exp)
        # (h2 copies follow on GpSimd; only needed by the vT matmuls)
        for t in range(CT):
            nc.gpsimd.tensor_scalar(
                out=h2[:, t // 2, :, t % 2, :], in0=xsb[:, t, :],
                scalar1=ab[:, t, 0:1], scalar2=ab[:, t, 1:2],
                op0=OP.mult, op1=OP.add,
            )

        # ---- q / k projections, DoubleRow over c-tile pairs ----
        # q plain [c, hw] (moving in S^T); k pair-contiguous (stationary)
        q_sb = big.tile([P, CT, HW], F8, tag="q")
        k2 = big.tile([P, CP, JT, 2, P], F8, tag="k2")
        for ot in range(CT):
            for ib in range(IB):
                isl = slice(ib * NB, (ib + 1) * NB)
                psq = ps_pool.tile([P, NB], F32, tag="mmps")
                for cp in range(CP):
                    nc.tensor.matmul(
                        psq[:], wq_sb[:, cp, ot, :, :],
                        hsb[:, 2 * cp : 2 * cp + 2, isl],
                        start=(cp == 0), stop=(cp == CP - 1), perf_mode=DR,
                    )
                nc.scalar.activation(
                    out=q_sb[:, ot, isl], in_=psq[:],
                    func=AF.Identity, bias=bq_sb[:, ot : ot + 1],
                )
                psk = ps_pool.tile([P, NB], F32, tag="mmps")
                for cp in range(CP):
                    nc.tensor.matmul(
                        psk[:], wk_sb[:, cp, ot, :, :],
                        hsb[:, 2 * cp : 2 * cp + 2, isl],
                        start=(cp == 0), stop=(cp == CP - 1), perf_mode=DR,
                    )
                nc.vector.tensor_scalar_add(
                    k2[:, ot // 2, 4 * ib : 4 * ib + 4, ot % 2, :],
                    psk[:], bk_sb[:, ot : ot + 1],
                )

        # ---- vT projection; pure copy drains split ACT/DVE ----
        vT2 = big.tile([P, JP, CT, 2, P], F8, tag="vT2")
        for jt in range(JT):
            psv = ps_pool.tile([P, NB], F32, tag="mmps")
            for cp in range(CP):
                nc.tensor.matmul(
                    psv[:], h2[:, cp, jt, :, :], wv_sb[:, 2 * cp : 2 * cp + 2, :],
                    start=(cp == 0), stop=(cp == CP - 1), perf_mode=DR,
                )
            if jt % 2 == 0:
                nc.scalar.activation(
                    out=vT2[:, jt // 2, :, jt % 2, :], in_=psv[:], func=AF.Identity
                )
            else:
                nc.vector.tensor_copy(out=vT2[:, jt // 2, :, jt % 2, :], in_=psv[:])

        # ---- S^T + exp + denominator for BOTH query blocks ----
        rep = big.tile([P, HW], F32, tag="rep")
        u_sb = big.tile([P, CT, HW], F8, tag="u")
        ps_ds = []
        for ib in range(IB):
            isl = slice(ib * NB, (ib + 1) * NB)
            ps_d = ps_pool.tile([P, NB], F32, tag="mmps", name=f"psd{ib}")
            for jp in range(JP):
                pss = ps_pair.tile([P, 2, NB], F32, tag="sps")
                for h in range(2):
                    for cp in range(CP):
                        nc.tensor.matmul(
                            pss[:, h, :], k2[:, cp, 2 * jp + h, :, :],
                            q_sb[:, 2 * cp : 2 * cp + 2, isl],
                            start=(cp == 0), stop=(cp == CP - 1), perf_mode=DR,
                        )
                # attention scale rides the exp; fp8 est for both blocks
                nc.scalar.activation(
                    out=est[:, 2 * jp : 2 * jp + 2, isl], in_=pss[:],
                    func=AF.Exp, scale=SC,
                )
                # denom[i] += est pair, broadcast to all 128 partitions
                nc.tensor.matmul(
                    ps_d[:], ones8[:], est[:, 2 * jp : 2 * jp + 2, isl],
                    start=(jp == 0), stop=(jp == JP - 1), perf_mode=DR,
                )
            ps_ds.append(ps_d)
        # split reciprocals across engines so they run in parallel: block 0
        # on DVE (starts right after den0, overlapped by the S1 matmuls),
        # block 1 as exp(-ln d) on ACT right after the est exps
        nc.vector.reciprocal(out=rep[:, 0:NB], in_=ps_ds[0][:])
        nc.scalar.activation(out=rep[:, NB:HW], in_=ps_ds[1][:], func=AF.Ln)
        nc.scalar.activation(
            out=rep[:, NB:HW], in_=rep[:, NB:HW], func=AF.Exp, scale=-1.0
        )

        # ---- per block: U (covers the reciprocal), then proj + out DMA ----
        outr = out.rearrange("(t p) i -> p t i", p=P)
        for ib in range(IB):
            isl = slice(ib * NB, (ib + 1) * NB)
            for ct in range(CT):
                psu = ps_pool.tile([P, NB], F32, tag="mmps", name="psu")
                for jp in range(JP):
                    nc.tensor.matmul(
                        psu[:], vT2[:, jp, ct, :, :],
                        est[:, 2 * jp : 2 * jp + 2, isl],
                        start=(jp == 0), stop=(jp == JP - 1), perf_mode=DR,
                    )
                nc.vector.tensor_tensor(
                    out=u_sb[:, ct, isl], in0=psu[:], in1=rep[:, isl], op=OP.mult
                )
            for ot in range(CT):
                psp = ps_pool.tile([P, NB], F32, tag="mmps")
                for cp in range(CP):
                    nc.tensor.matmul(
                        psp[:], wp_sb[:, cp, ot, :, :],
                        u_sb[:, 2 * cp : 2 * cp + 2, isl],
                        start=(cp == 0), stop=(cp == CP - 1), perf_mode=DR,
                    )
                ot_t = outp.tile([P, NB], F32, tag="out", name="ot_t")
                if ot % 2 == 0:
                    nc.vector.scalar_tensor_tensor(
                        out=ot_t[:], in0=psp[:], scalar=bp_sb[:, ot : ot + 1],
                        in1=xsb[:, ot, isl], op0=OP.add, op1=OP.add,
                    )
                else:
                    # ACT (Identity, no table) + DVE so drains keep pace
                    yb = outp.tile([P, NB], F32, tag="out", name="yb")
                    nc.scalar.activation(
                        out=yb[:], in_=psp[:], func=AF.Identity,
                        bias=bp_sb[:, ot : ot + 1],
                    )
                    nc.vector.tensor_tensor(
                        out=ot_t[:], in0=yb[:], in1=xsb[:, ot, isl], op=OP.add
                    )
                # one dma_start per tile (it stripes across all 16 queues),
                # all on the sync queue: fewer completion semaphores for the
                # final barrier to sweep
                nc.sync.dma_start(out=outr[:, ot, isl], in_=ot_t[:])
    return nc


_NC = None


def _get_nc():
    global _NC
    if _NC is None:
        _NC = _build()
    return _NC


def _prep_inputs(x, gn_scale, gn_bias, wq, bq, wk, bk, wv, bv, wproj, bproj):
    f = np.float32
    x = np.ascontiguousarray(x, dtype=f).reshape(B, C, HW)

    def t8(w):  # [o, c] -> [c, o] fp8
        return np.ascontiguousarray(np.asarray(w, dtype=f).T).astype(NP8)

    def w2(w):  # [o, c] -> [p, (cp ot h o')] dual-fp8 ldweights layout
        wT = np.asarray(w, dtype=f).T  # [c, o]
        a = wT.reshape(CP, 2, P, CT, P)  # [cp, h, p, ot, o']
        a = a.transpose(2, 0, 3, 1, 4).reshape(P, CP * CT * 2 * P)
        return np.ascontiguousarray(a).astype(NP8)

    def pt(v):  # [512] -> [128, 4] with v[t*128 + p] at [p, t]
        return np.ascontiguousarray(np.asarray(v, dtype=f).reshape(CT, P).T)

    pidx = np.arange(P)[:, None]
    tidx = np.arange(CT)[None, :]
    grp = 2 * tidx + pidx // GS  # [128, 4] group id per (p, t)
    gmat = np.zeros((P, CT, NG), f)
    hmat = np.zeros((P, CT, P), f)
    for t in range(CT):
        gmat[pidx[:, 0], t, grp[:, t]] = 1.0
        hmat[grp[:, t], t, pidx[:, 0]] = 1.0

    bp_eff = np.asarray(bproj, f) + np.asarray(wproj, f) @ np.asarray(bv, f)
    smallcat = np.concatenate(
        [
            pt(bq), pt(bk), pt(bp_eff), pt(gn_scale), pt(gn_bias),
            gmat.reshape(P, CT * NG), hmat.reshape(P, CT * P),
        ],
        axis=1,
    )
    shared = {
        "wq2": w2(wq), "wk2": w2(wk), "wp2": w2(wproj), "wvT": t8(wv),
        "smallcat": np.ascontiguousarray(smallcat),
    }
    return [dict(shared, x=np.ascontiguousarray(x[b])) for b in range(B)]


def _run(inputs, **kw):
    nc = _get_nc()
    in_maps = _prep_inputs(**inputs)
    return run_bass_kernel_spmd(nc, in_maps, core_ids=list(range(B)), **kw)


def kernel(**inputs) -> np.ndarray:
    res = _run(inputs)
    out = np.stack([res.results[b]["out"] for b in range(B)])
    return out.reshape(B, C, HH, WW).astype(np.float32)
